# revision 8
# baseline (speedup 1.0000x reference)
"""EnergyAttention Trainium2 kernel (8-core SPMD, head/q hybrid sharding).

reference math:
    K = einsum('kd,hzd->khz', g, Wk); Q = einsum('qd,hzd->qhz', g, Wq)
    scores = beta * einsum('qhz,khz->hqk', Q, K)        # [H, N, N]
    A = logsumexp(scores, -1); out = (-1/beta) * A.sum()

Sharding (no collectives; final scalar reduction on host):
    core c owns head A = c (all 2048 q rows) and head B = 8 + c//2
    restricted to q rows [1024*(c%2), 1024*(c%2)+1024).  Every core runs an
    identical program; only input values differ (SPMD-safe).

Per core the kernel computes, for 24 q-tiles of 128 rows, the per-row
neg-max and sum-exp of the 2048-wide score rows; the host finishes with
A = m + log(l) and the fp64 sum.  beta is folded into Wq on the host.
"""

import numpy as np
from contextlib import ExitStack

import concourse.bass as bass
import concourse.mybir as mybir
import concourse.tile as tile
from concourse import bacc
from concourse.bass_utils import run_bass_kernel_spmd
from concourse.masks import make_identity

N, D, H, Y = 2048, 768, 12, 64
NCORES = 8
BETA = 1.0 / 8.0
DT = mybir.dt.float32
FP32R = False  # flip to use float32r matmuls (4x PE throughput at free>=256)


def _units():
    # interleave A and B q-tile units so adjacent PE matmuls use disjoint
    # row groups (A contracts on partitions 0:64, B on 64:128)
    units = []
    for j in range(8):
        units.append(("A", j))
        units.append(("B", j))
    for j in range(8, 16):
        units.append(("A", j))
    return units


def _mm(x):
    return x.bitcast(mybir.dt.float32r) if FP32R else x


def _build_kernel():
    nc = bacc.Bacc("TRN2", target_bir_lowering=False, debug=False, num_devices=1)
    g_ap = nc.dram_tensor("g", [N, D], DT, kind="ExternalInput").ap()
    gb_ap = nc.dram_tensor("gb", [N // 2, D], DT, kind="ExternalInput").ap()
    wq_ap = nc.dram_tensor("wq", [128, D], DT, kind="ExternalInput").ap()
    wk_ap = nc.dram_tensor("wk", [128, D], DT, kind="ExternalInput").ap()
    out_ap = nc.dram_tensor("stats", [128, 48], DT, kind="ExternalOutput").ap()

    AF = mybir.ActivationFunctionType
    AX = mybir.AxisListType
    OP = mybir.AluOpType

    with tile.TileContext(nc) as tc, ExitStack() as ctx:
        const_pool = ctx.enter_context(tc.tile_pool(name="const", bufs=1))
        ident = const_pool.tile([128, 128], DT)
        make_identity(nc, ident[:])

        w_pool = ctx.enter_context(tc.tile_pool(name="w", bufs=1))
        wq_sb = w_pool.tile([128, D], DT)
        nc.sync.dma_start(wq_sb[:], wq_ap[:])
        wk_sb = w_pool.tile([128, D], DT)
        nc.sync.dma_start(wk_sb[:], wk_ap[:])
        # wt blocks 0..5 = WqT d-tiles ([128d, 64 zA | 64 zB]), 6..11 = WkT
        wt_sb = w_pool.tile([128, 12 * 128], DT)

        proj_pool = ctx.enter_context(tc.tile_pool(name="proj", bufs=1))
        kt_sb = proj_pool.tile([128, N], DT)       # rows 0:64 KT_A, 64:128 KT_B
        qta_sb = proj_pool.tile([64, N], DT)       # QT of head A, all q
        qtb_sb = proj_pool.tile([128, N // 2], DT)  # rows 64:128 = QT of head B
        stat_pool = ctx.enter_context(tc.tile_pool(name="stat", bufs=8))

        with tc.tile_pool(name="gsrc", bufs=1) as gsrc_pool, \
             tc.tile_pool(name="gt", bufs=1) as gt_pool:
          with tc.tile_pool(name="tp", bufs=6, space="PSUM") as tp:

            # ---- W transposes: 12 [128,128] blocks, batched 4 per PSUM bank
            for grp in range(3):
                ps = tp.tile([128, 512], DT, tag="tps", name="ps_w")
                for j in range(4):
                    blk = grp * 4 + j
                    src = wq_sb if blk < 6 else wk_sb
                    t = blk % 6
                    nc.tensor.transpose(
                        ps[:, 128 * j : 128 * (j + 1)],
                        src[:, 128 * t : 128 * (t + 1)],
                        ident[:],
                    )
                nc.vector.tensor_copy(wt_sb[:, 512 * grp : 512 * (grp + 1)], ps[:])

            # ---- load g / gb (chunked so transposes can start early)
            g_sb = gsrc_pool.tile([128, 16, D], DT)
            g_r = g_ap.rearrange("(i p) d -> p i d", p=128)
            for c in range(4):
                nc.sync.dma_start(g_sb[:, 4 * c : 4 * (c + 1), :], g_r[:, 4 * c : 4 * (c + 1), :])
            gb_sb = gsrc_pool.tile([128, 8, D], DT)
            gb_r = gb_ap.rearrange("(i p) d -> p i d", p=128)
            for c in range(2):
                nc.sync.dma_start(gb_sb[:, 4 * c : 4 * (c + 1), :], gb_r[:, 4 * c : 4 * (c + 1), :])

            # ---- transpose g -> gT [6 d-tiles x (128, 2048)], gb likewise
            gt_sb = gt_pool.tile([128, 6, N], DT)
            gtb_sb = gt_pool.tile([128, 6, N // 2], DT)
            for c in range(4):
                for t in range(6):
                    ps = tp.tile([128, 512], DT, tag="tps", name="ps_g")
                    for j in range(4):
                        i = 4 * c + j
                        nc.tensor.transpose(
                            ps[:, 128 * j : 128 * (j + 1)],
                            g_sb[:, i, 128 * t : 128 * (t + 1)],
                            ident[:],
                        )
                    nc.vector.tensor_copy(gt_sb[:, t, 512 * c : 512 * (c + 1)], ps[:])
            for c in range(2):
                for t in range(6):
                    ps = tp.tile([128, 512], DT, tag="tps", name="ps_gb")
                    for j in range(4):
                        i = 4 * c + j
                        nc.tensor.transpose(
                            ps[:, 128 * j : 128 * (j + 1)],
                            gb_sb[:, i, 128 * t : 128 * (t + 1)],
                            ident[:],
                        )
                    nc.vector.tensor_copy(gtb_sb[:, t, 512 * c : 512 * (c + 1)], ps[:])

          # ---- projections (PSUM accumulate over 6 d-tiles)
          with tc.tile_pool(name="pp", bufs=2, space="PSUM") as pp:
                for c in range(4):  # KT, both heads packed on 128 partitions
                    ps = pp.tile([128, 512], DT, tag="ppk", name="ps_kt")
                    for t in range(6):
                        nc.tensor.matmul(
                            ps[:],
                            lhsT=_mm(wt_sb[:, 128 * (6 + t) : 128 * (7 + t)]),
                            rhs=_mm(gt_sb[:, t, 512 * c : 512 * (c + 1)]),
                            start=(t == 0),
                            stop=(t == 5),
                        )
                    nc.scalar.copy(kt_sb[:, 512 * c : 512 * (c + 1)], ps[:])
                for c in range(2):  # QT of head B -> partitions 64:128
                    ps = pp.tile([128, 512], DT, tag="ppb", name="ps_qtb")
                    for t in range(6):
                        nc.tensor.matmul(
                            ps[64:128, :],
                            lhsT=_mm(wt_sb[:, 128 * t + 64 : 128 * (t + 1)]),
                            rhs=_mm(gtb_sb[:, t, 512 * c : 512 * (c + 1)]),
                            start=(t == 0),
                            stop=(t == 5),
                            tile_position=(0, 64),
                        )
                    nc.scalar.copy(qtb_sb[64:128, 512 * c : 512 * (c + 1)], ps[64:128, :])
                for c in range(4):  # QT of head A -> partitions 0:64
                    ps = pp.tile([64, 512], DT, tag="ppa", name="ps_qta")
                    for t in range(6):
                        nc.tensor.matmul(
                            ps[:],
                            lhsT=_mm(wt_sb[:, 128 * t : 128 * t + 64]),
                            rhs=_mm(gt_sb[:, t, 512 * c : 512 * (c + 1)]),
                            start=(t == 0),
                            stop=(t == 5),
                        )
                    nc.scalar.copy(qta_sb[:, 512 * c : 512 * (c + 1)], ps[:])

        # ---- scores + logsumexp stats per 128-row q-tile
        with tc.tile_pool(name="sp", bufs=2, space="PSUM") as sp:
            for u, (kind, j) in enumerate(_units()):
                ps = sp.tile([128, N], DT, tag="sps", name="ps_s")
                for c in range(4):
                    if kind == "A":
                        lhsT = qta_sb[:, 128 * j : 128 * (j + 1)]
                        rhs = kt_sb[0:64, 512 * c : 512 * (c + 1)]
                    else:
                        lhsT = qtb_sb[64:128, 128 * j : 128 * (j + 1)]
                        rhs = kt_sb[64:128, 512 * c : 512 * (c + 1)]
                    nc.tensor.matmul(
                        ps[:, 512 * c : 512 * (c + 1)],
                        lhsT=_mm(lhsT),
                        rhs=_mm(rhs),
                        start=True,
                        stop=True,
                    )
                st = stat_pool.tile([128, 2], DT, tag="st", name="st")
                nc.vector.tensor_reduce(
                    st[:, 0:1], ps[:], axis=AX.X, op=OP.max, negate=True
                )
                nc.scalar.activation(
                    ps[:], ps[:], AF.Exp, bias=st[:, 0:1], scale=1.0,
                    accum_out=st[:, 1:2],
                )
                nc.sync.dma_start(out_ap[:, 2 * u : 2 * (u + 1)], st[:])

    nc.compile()
    return nc


_NC_CACHE = {}


def _get_nc():
    if "nc" not in _NC_CACHE:
        _NC_CACHE["nc"] = _build_kernel()
    return _NC_CACHE["nc"]


def _make_in_maps(np_inputs):
    g = np.ascontiguousarray(np.asarray(np_inputs["g"], dtype=np.float32))
    Wq = np.asarray(np_inputs["Wq"], dtype=np.float32)
    Wk = np.asarray(np_inputs["Wk"], dtype=np.float32)
    in_maps = []
    for c in range(NCORES):
        hb = 8 + c // 2
        qlo = (N // 2) * (c % 2)
        in_maps.append(
            {
                "g": g,
                "gb": np.ascontiguousarray(g[qlo : qlo + N // 2]),
                "wq": np.ascontiguousarray(
                    np.concatenate([Wq[c], Wq[hb]], axis=0) * np.float32(BETA)
                ),
                "wk": np.ascontiguousarray(np.concatenate([Wk[c], Wk[hb]], axis=0)),
            }
        )
    return in_maps


def kernel(g, Wq, Wk):
    in_maps = _make_in_maps({"g": g, "Wq": Wq, "Wk": Wk})
    nc = _get_nc()
    res = run_bass_kernel_spmd(nc, in_maps, core_ids=list(range(NCORES)))

    total = 0.0
    for c in range(NCORES):
        stats = res.results[c]["stats"].astype(np.float64)  # [128, 48]
        neg_m = stats[:, 0::2]  # [128, 24]
        l = stats[:, 1::2]
        total += (-neg_m + np.log(l)).sum()
    return np.float32(-(1.0 / BETA) * total)


# revision 11
# speedup vs baseline: 1.6755x; 1.6755x over previous
"""EnergyAttention Trainium2 kernel (8-core SPMD, head/q hybrid sharding).

reference math:
    K = einsum('kd,hzd->khz', g, Wk); Q = einsum('qd,hzd->qhz', g, Wq)
    scores = beta * einsum('qhz,khz->hqk', Q, K)        # [H, N, N]
    A = logsumexp(scores, -1); out = (-1/beta) * A.sum()

Sharding (no collectives; final scalar reduction on host):
    core c owns head A = c (all 2048 q rows) and head B = 8 + c//2
    restricted to q rows [1024*(c%2), 1024*(c%2)+1024).  Every core runs an
    identical program; only input values differ (SPMD-safe).

Per core the kernel computes, for 24 q-tiles of 128 rows, the per-row
neg-max and sum-exp of the 2048-wide score rows; the host finishes with
A = m + log(l) and the fp64 sum.  beta is folded into Wq on the host.
"""

import numpy as np
from contextlib import ExitStack

import concourse.bass as bass
import concourse.mybir as mybir
import concourse.tile as tile
from concourse import bacc
from concourse.bass_utils import run_bass_kernel_spmd
from concourse.masks import make_identity

N, D, H, Y = 2048, 768, 12, 64
NCORES = 8
BETA = 1.0 / 8.0
DT = mybir.dt.float32
FP32R = True  # float32r matmuls: 4x PE throughput at free>=256
DTM = mybir.dt.float32r if FP32R else mybir.dt.float32  # matmul-operand tiles


def _units():
    # interleave A and B q-tile units so adjacent PE matmuls use disjoint
    # row groups (A contracts on partitions 0:64, B on 64:128)
    units = []
    for j in range(8):
        units.append(("A", j))
        units.append(("B", j))
    for j in range(8, 16):
        units.append(("A", j))
    return units


def _mm(x):
    return x


def _build_kernel():
    nc = bacc.Bacc("TRN2", target_bir_lowering=False, debug=False, num_devices=1)
    g_ap = nc.dram_tensor("g", [N, D], DT, kind="ExternalInput").ap()
    gb_ap = nc.dram_tensor("gb", [N // 2, D], DT, kind="ExternalInput").ap()
    wq_ap = nc.dram_tensor("wq", [128, D], DT, kind="ExternalInput").ap()
    wk_ap = nc.dram_tensor("wk", [128, D], DT, kind="ExternalInput").ap()
    out_ap = nc.dram_tensor("stats", [128, 48], DT, kind="ExternalOutput").ap()

    AF = mybir.ActivationFunctionType
    AX = mybir.AxisListType
    OP = mybir.AluOpType

    with tile.TileContext(nc) as tc, ExitStack() as ctx:
        const_pool = ctx.enter_context(tc.tile_pool(name="const", bufs=1))
        ident = const_pool.tile([128, 128], DT)
        make_identity(nc, ident[:])

        w_pool = ctx.enter_context(tc.tile_pool(name="w", bufs=1))
        wq_sb = w_pool.tile([128, D], DT)
        nc.sync.dma_start(wq_sb[:], wq_ap[:])
        wk_sb = w_pool.tile([128, D], DT)
        nc.sync.dma_start(wk_sb[:], wk_ap[:])
        # wt blocks 0..5 = WqT d-tiles ([128d, 64 zA | 64 zB]), 6..11 = WkT
        wt_sb = w_pool.tile([128, 12 * 128], DTM)

        proj_pool = ctx.enter_context(tc.tile_pool(name="proj", bufs=1))
        kt_sb = proj_pool.tile([128, N], DTM)       # rows 0:64 KT_A, 64:128 KT_B
        qta_sb = proj_pool.tile([64, N], DTM)       # QT of head A, all q
        qtb_sb = proj_pool.tile([128, N // 2], DTM)  # rows 64:128 = QT of head B
        stat_pool = ctx.enter_context(tc.tile_pool(name="stat", bufs=8))

        with tc.tile_pool(name="gsrc", bufs=1) as gsrc_pool, \
             tc.tile_pool(name="gt", bufs=1) as gt_pool:
          with tc.tile_pool(name="tp", bufs=6, space="PSUM") as tp:

            # ---- W transposes: 12 [128,128] blocks, batched 4 per PSUM bank
            for grp in range(3):
                ps = tp.tile([128, 512], DT, tag="tps", name="ps_w")
                for j in range(4):
                    blk = grp * 4 + j
                    src = wq_sb if blk < 6 else wk_sb
                    t = blk % 6
                    nc.tensor.transpose(
                        ps[:, 128 * j : 128 * (j + 1)],
                        src[:, 128 * t : 128 * (t + 1)],
                        ident[:],
                    )
                nc.vector.tensor_copy(wt_sb[:, 512 * grp : 512 * (grp + 1)], ps[:])

            # ---- load g / gb (chunked so transposes can start early)
            g_sb = gsrc_pool.tile([128, 16, D], DT)
            g_r = g_ap.rearrange("(i p) d -> p i d", p=128)
            for c in range(4):
                nc.sync.dma_start(g_sb[:, 4 * c : 4 * (c + 1), :], g_r[:, 4 * c : 4 * (c + 1), :])
            gb_sb = gsrc_pool.tile([128, 8, D], DT)
            gb_r = gb_ap.rearrange("(i p) d -> p i d", p=128)
            for c in range(2):
                nc.sync.dma_start(gb_sb[:, 4 * c : 4 * (c + 1), :], gb_r[:, 4 * c : 4 * (c + 1), :])

            # ---- transpose g -> gT [6 d-tiles x (128, 2048)], gb likewise
            gt_sb = gt_pool.tile([128, 6, N], DTM)
            gtb_sb = gt_pool.tile([128, 6, N // 2], DTM)
            for c in range(4):
                for t in range(6):
                    ps = tp.tile([128, 512], DT, tag="tps", name="ps_g")
                    for j in range(4):
                        i = 4 * c + j
                        nc.tensor.transpose(
                            ps[:, 128 * j : 128 * (j + 1)],
                            g_sb[:, i, 128 * t : 128 * (t + 1)],
                            ident[:],
                        )
                    nc.vector.tensor_copy(gt_sb[:, t, 512 * c : 512 * (c + 1)], ps[:])
            for c in range(2):
                for t in range(6):
                    ps = tp.tile([128, 512], DT, tag="tps", name="ps_gb")
                    for j in range(4):
                        i = 4 * c + j
                        nc.tensor.transpose(
                            ps[:, 128 * j : 128 * (j + 1)],
                            gb_sb[:, i, 128 * t : 128 * (t + 1)],
                            ident[:],
                        )
                    nc.vector.tensor_copy(gtb_sb[:, t, 512 * c : 512 * (c + 1)], ps[:])

          # ---- projections (PSUM accumulate over 6 d-tiles)
          with tc.tile_pool(name="pp", bufs=2, space="PSUM") as pp:
                for c in range(4):  # KT, both heads packed on 128 partitions
                    ps = pp.tile([128, 512], DT, tag="ppk", name="ps_kt")
                    for t in range(6):
                        nc.tensor.matmul(
                            ps[:],
                            lhsT=_mm(wt_sb[:, 128 * (6 + t) : 128 * (7 + t)]),
                            rhs=_mm(gt_sb[:, t, 512 * c : 512 * (c + 1)]),
                            start=(t == 0),
                            stop=(t == 5),
                        )
                    nc.scalar.copy(kt_sb[:, 512 * c : 512 * (c + 1)], ps[:])
                # QT of head B: fp32r matmuls cannot target col-group 64
                # (tile_position is invalid ISA with fp32r), so project at
                # partitions 0:64 and DMA-shift to partitions 64:128.
                qtb_lo = proj_pool.tile([64, N // 2], DTM)
                for c in range(2):
                    ps = pp.tile([64, 512], DT, tag="ppb", name="ps_qtb")
                    for t in range(6):
                        nc.tensor.matmul(
                            ps[:],
                            lhsT=_mm(wt_sb[:, 128 * t + 64 : 128 * (t + 1)]),
                            rhs=_mm(gtb_sb[:, t, 512 * c : 512 * (c + 1)]),
                            start=(t == 0),
                            stop=(t == 5),
                        )
                    nc.scalar.copy(qtb_lo[:, 512 * c : 512 * (c + 1)], ps[:])
                nc.sync.dma_start(qtb_sb[64:128, :], qtb_lo[:])
                for c in range(4):  # QT of head A -> partitions 0:64
                    ps = pp.tile([64, 512], DT, tag="ppa", name="ps_qta")
                    for t in range(6):
                        nc.tensor.matmul(
                            ps[:],
                            lhsT=_mm(wt_sb[:, 128 * t : 128 * t + 64]),
                            rhs=_mm(gt_sb[:, t, 512 * c : 512 * (c + 1)]),
                            start=(t == 0),
                            stop=(t == 5),
                        )
                    nc.scalar.copy(qta_sb[:, 512 * c : 512 * (c + 1)], ps[:])

        # ---- scores + logsumexp stats per 128-row q-tile
        with tc.tile_pool(name="sp", bufs=2, space="PSUM") as sp:
            for u, (kind, j) in enumerate(_units()):
                ps = sp.tile([128, N], DT, tag="sps", name="ps_s")
                for c in range(4):
                    if kind == "A":
                        lhsT = qta_sb[:, 128 * j : 128 * (j + 1)]
                        rhs = kt_sb[0:64, 512 * c : 512 * (c + 1)]
                    else:
                        lhsT = qtb_sb[64:128, 128 * j : 128 * (j + 1)]
                        rhs = kt_sb[64:128, 512 * c : 512 * (c + 1)]
                    nc.tensor.matmul(
                        ps[:, 512 * c : 512 * (c + 1)],
                        lhsT=_mm(lhsT),
                        rhs=_mm(rhs),
                        start=True,
                        stop=True,
                    )
                st = stat_pool.tile([128, 2], DT, tag="st", name="st")
                nc.vector.tensor_reduce(
                    st[:, 0:1], ps[:], axis=AX.X, op=OP.max, negate=True
                )
                nc.scalar.activation(
                    ps[:], ps[:], AF.Exp, bias=st[:, 0:1], scale=1.0,
                    accum_out=st[:, 1:2],
                )
                nc.sync.dma_start(out_ap[:, 2 * u : 2 * (u + 1)], st[:])

    nc.compile()
    return nc


_NC_CACHE = {}


def _get_nc():
    if "nc" not in _NC_CACHE:
        _NC_CACHE["nc"] = _build_kernel()
    return _NC_CACHE["nc"]


def _make_in_maps(np_inputs):
    g = np.ascontiguousarray(np.asarray(np_inputs["g"], dtype=np.float32))
    Wq = np.asarray(np_inputs["Wq"], dtype=np.float32)
    Wk = np.asarray(np_inputs["Wk"], dtype=np.float32)
    in_maps = []
    for c in range(NCORES):
        hb = 8 + c // 2
        qlo = (N // 2) * (c % 2)
        in_maps.append(
            {
                "g": g,
                "gb": np.ascontiguousarray(g[qlo : qlo + N // 2]),
                "wq": np.ascontiguousarray(
                    np.concatenate([Wq[c], Wq[hb]], axis=0) * np.float32(BETA)
                ),
                "wk": np.ascontiguousarray(np.concatenate([Wk[c], Wk[hb]], axis=0)),
            }
        )
    return in_maps


def kernel(g, Wq, Wk):
    in_maps = _make_in_maps({"g": g, "Wq": Wq, "Wk": Wk})
    nc = _get_nc()
    res = run_bass_kernel_spmd(nc, in_maps, core_ids=list(range(NCORES)))

    total = 0.0
    for c in range(NCORES):
        stats = res.results[c]["stats"].astype(np.float64)  # [128, 48]
        neg_m = stats[:, 0::2]  # [128, 24]
        l = stats[:, 1::2]
        total += (-neg_m + np.log(l)).sum()
    return np.float32(-(1.0 / BETA) * total)


# revision 12
# speedup vs baseline: 1.9959x; 1.1912x over previous
"""EnergyAttention Trainium2 kernel (8-core SPMD, head/q hybrid sharding).

reference math:
    K = einsum('kd,hzd->khz', g, Wk); Q = einsum('qd,hzd->qhz', g, Wq)
    scores = beta * einsum('qhz,khz->hqk', Q, K)        # [H, N, N]
    A = logsumexp(scores, -1); out = (-1/beta) * A.sum()

Sharding (no collectives; final scalar reduction on host):
    core c owns head A = c (all 2048 q rows) and head B = 8 + c//2
    restricted to q rows [1024*(c%2), 1024*(c%2)+1024).  Every core runs an
    identical program; only input values differ (SPMD-safe).

Implementation notes:
  - inputs are cast to bf16 on the host (beta folded into Wq); matmuls are
    bf16 with fp32 PSUM accumulation
  - all transposes (g -> gT, W -> WT) ride the DMA crossbar transpose
    straight out of DRAM -- zero engine cost
  - per 128-row q-tile: 4 matmuls -> PSUM scores [128, 2048]; DVE
    reduce_max(negate) -> ACT exp(bias=-m) with fused accum row-sum
  - host finishes: A = m + log(l), fp64 sum, scale by -1/beta
"""

import numpy as np
import ml_dtypes
from contextlib import ExitStack

import concourse.bass as bass
import concourse.mybir as mybir
import concourse.tile as tile
from concourse import bacc
from concourse.bass_utils import run_bass_kernel_spmd

N, D, H, Y = 2048, 768, 12, 64
NCORES = 8
BETA = 1.0 / 8.0
DT = mybir.dt.float32
DTB = mybir.dt.bfloat16


def _units():
    # interleave A and B q-tile units so adjacent PE matmuls use disjoint
    # row groups (A contracts on partitions 0:64, B on 64:128)
    units = []
    for j in range(8):
        units.append(("A", j))
        units.append(("B", j))
    for j in range(8, 16):
        units.append(("A", j))
    return units


def _build_kernel():
    nc = bacc.Bacc("TRN2", target_bir_lowering=False, debug=False, num_devices=1)
    g_ap = nc.dram_tensor("g", [N, D], DTB, kind="ExternalInput").ap()
    gb_ap = nc.dram_tensor("gb", [N // 2, D], DTB, kind="ExternalInput").ap()
    wq_ap = nc.dram_tensor("wq", [128, D], DTB, kind="ExternalInput").ap()
    wk_ap = nc.dram_tensor("wk", [128, D], DTB, kind="ExternalInput").ap()
    out_ap = nc.dram_tensor("stats", [128, 48], DT, kind="ExternalOutput").ap()

    AF = mybir.ActivationFunctionType
    AX = mybir.AxisListType
    OP = mybir.AluOpType

    with tile.TileContext(nc) as tc, ExitStack() as ctx:
        w_pool = ctx.enter_context(tc.tile_pool(name="w", bufs=1))
        # WT layout per d-tile t: [128 d, 64 zA | 64 zB]
        wt_q = w_pool.tile([128, 6, 128], DTB)
        nc.sync.dma_start_transpose(wt_q[:], wq_ap[:])
        wt_k = w_pool.tile([128, 6, 128], DTB)
        nc.sync.dma_start_transpose(wt_k[:], wk_ap[:])

        proj_pool = ctx.enter_context(tc.tile_pool(name="proj", bufs=1))
        kt_sb = proj_pool.tile([128, N], DTB)       # rows 0:64 KT_A, 64:128 KT_B
        qta_sb = proj_pool.tile([64, N], DTB)       # QT of head A, all q
        qtb_sb = proj_pool.tile([128, N // 2], DTB)  # rows 64:128 = QT of head B
        stat_pool = ctx.enter_context(tc.tile_pool(name="stat", bufs=8))

        with tc.tile_pool(name="gt", bufs=1) as gt_pool:
            # gT via xbar transpose straight from DRAM:
            # gt[c][p, t, i] = g[512c + i, 128t + p]
            gt = []
            for c in range(4):
                gtc = gt_pool.tile([128, 6, 512], DTB, name=f"gt{c}")
                nc.sync.dma_start_transpose(gtc[:], g_ap[512 * c : 512 * (c + 1), :])
                gt.append(gtc)
            gtb = []
            for c in range(2):
                gtbc = gt_pool.tile([128, 6, 512], DTB, name=f"gtb{c}")
                nc.sync.dma_start_transpose(gtbc[:], gb_ap[512 * c : 512 * (c + 1), :])
                gtb.append(gtbc)

            # ---- projections (PSUM accumulate over 6 d-tiles)
            with tc.tile_pool(name="pp", bufs=2, space="PSUM") as pp:
                for c in range(4):  # KT, both heads packed on 128 partitions
                    ps = pp.tile([128, 512], DT, tag="ppk", name="ps_kt")
                    for t in range(6):
                        nc.tensor.matmul(
                            ps[:],
                            lhsT=wt_k[:, t, :],
                            rhs=gt[c][:, t, :],
                            start=(t == 0),
                            stop=(t == 5),
                        )
                    nc.scalar.copy(kt_sb[:, 512 * c : 512 * (c + 1)], ps[:])
                # QT of head B at partitions 0:64, then DMA-shift to 64:128
                qtb_lo = proj_pool.tile([64, N // 2], DTB)
                for c in range(2):
                    ps = pp.tile([64, 512], DT, tag="ppb", name="ps_qtb")
                    for t in range(6):
                        nc.tensor.matmul(
                            ps[:],
                            lhsT=wt_q[:, t, 64:128],
                            rhs=gtb[c][:, t, :],
                            start=(t == 0),
                            stop=(t == 5),
                        )
                    nc.scalar.copy(qtb_lo[:, 512 * c : 512 * (c + 1)], ps[:])
                nc.sync.dma_start(qtb_sb[64:128, :], qtb_lo[:])
                for c in range(4):  # QT of head A -> partitions 0:64
                    ps = pp.tile([64, 512], DT, tag="ppa", name="ps_qta")
                    for t in range(6):
                        nc.tensor.matmul(
                            ps[:],
                            lhsT=wt_q[:, t, 0:64],
                            rhs=gt[c][:, t, :],
                            start=(t == 0),
                            stop=(t == 5),
                        )
                    nc.scalar.copy(qta_sb[:, 512 * c : 512 * (c + 1)], ps[:])

        # ---- scores + logsumexp stats per 128-row q-tile
        with tc.tile_pool(name="sp", bufs=2, space="PSUM") as sp:
            for u, (kind, j) in enumerate(_units()):
                ps = sp.tile([128, N], DT, tag="sps", name="ps_s")
                for c in range(4):
                    if kind == "A":
                        lhsT = qta_sb[:, 128 * j : 128 * (j + 1)]
                        rhs = kt_sb[0:64, 512 * c : 512 * (c + 1)]
                    else:
                        lhsT = qtb_sb[64:128, 128 * j : 128 * (j + 1)]
                        rhs = kt_sb[64:128, 512 * c : 512 * (c + 1)]
                    nc.tensor.matmul(
                        ps[:, 512 * c : 512 * (c + 1)],
                        lhsT=lhsT,
                        rhs=rhs,
                        start=True,
                        stop=True,
                    )
                st = stat_pool.tile([128, 2], DT, tag="st", name="st")
                nc.vector.tensor_reduce(
                    st[:, 0:1], ps[:], axis=AX.X, op=OP.max, negate=True
                )
                nc.scalar.activation(
                    ps[:], ps[:], AF.Exp, bias=st[:, 0:1], scale=1.0,
                    accum_out=st[:, 1:2],
                )
                nc.sync.dma_start(out_ap[:, 2 * u : 2 * (u + 1)], st[:])

    nc.compile()
    return nc


_NC_CACHE = {}


def _get_nc():
    if "nc" not in _NC_CACHE:
        _NC_CACHE["nc"] = _build_kernel()
    return _NC_CACHE["nc"]


def _make_in_maps(np_inputs):
    bf16 = ml_dtypes.bfloat16
    g = np.ascontiguousarray(np.asarray(np_inputs["g"], dtype=np.float32).astype(bf16))
    Wq = np.asarray(np_inputs["Wq"], dtype=np.float32) * np.float32(BETA)
    Wk = np.asarray(np_inputs["Wk"], dtype=np.float32)
    in_maps = []
    for c in range(NCORES):
        hb = 8 + c // 2
        qlo = (N // 2) * (c % 2)
        in_maps.append(
            {
                "g": g,
                "gb": np.ascontiguousarray(g[qlo : qlo + N // 2]),
                "wq": np.ascontiguousarray(
                    np.concatenate([Wq[c], Wq[hb]], axis=0).astype(bf16)
                ),
                "wk": np.ascontiguousarray(
                    np.concatenate([Wk[c], Wk[hb]], axis=0).astype(bf16)
                ),
            }
        )
    return in_maps


def kernel(g, Wq, Wk):
    in_maps = _make_in_maps({"g": g, "Wq": Wq, "Wk": Wk})
    nc = _get_nc()
    res = run_bass_kernel_spmd(nc, in_maps, core_ids=list(range(NCORES)))

    total = 0.0
    for c in range(NCORES):
        stats = res.results[c]["stats"].astype(np.float64)  # [128, 48]
        neg_m = stats[:, 0::2]  # [128, 24]
        l = stats[:, 1::2]
        total += (-neg_m + np.log(l)).sum()
    return np.float32(-(1.0 / BETA) * total)
